# revision 15
# baseline (speedup 1.0000x reference)
"""Trainium2 kernel for nn_AdaptedGNN (retrieval_knn affinity).

affinity[r, f] = (nf[2+f,2] + nf[2+f,4] + eps) / (dist(robot_r, frontier_f) + eps)

Fully data-parallel across 8 NeuronCores: core c owns frontier rows
[c*1e6, (c+1)*1e6), padded to 128*7824 = 1,001,472 rows.

v2 structure — minimize HBM bytes/row (the wall is the ~358 GB/s
HBM-per-core limit), then balance engines:

  DMA in : 6 B/row  — x,y as u16 fixed-point (x*65536), gain=f2+f4+eps fp16
           (host-precomputed). [x u16 | g fp16] interleaved plane + separate
           y plane (two f32/u16 streams from ONE tile stall the DVE).
  VectorE: S_r = (xu + cx_r)^2 + (yu + cy_r)^2  [DIST2 custom, u16 streams
           converted on input; cx_r = -rx*65536 per-partition AP; 1x mode]
           O_r = R_r * G                        [stock tensor_tensor fp16*fp16
           -> bf16, all-2-byte + aligned => 2x_1p mode]
  ScalarE: R_r = Rsqrt(S_r * 2^-32) = 1/dist    [raw ACT Rsqrt (accuracy
           guard bypassed; measured ~5e-4), scale folds the u16 rescale;
           one act table total. fp16 out keeps the TT in 2x mode.]
  DMA out: 4 B/row — 2x bf16 on the ACT HWDGE ring.

Host patches rows within PATCH_T of a robot (u16 position quantization error
~1.1e-5 abs would exceed the rel-err gate only when dist is tiny): ~160
rows/robot recomputed exactly in numpy. Division-by-~0 on device can yield
inf for those rows; all such rows are inside the patch radius.

Error budget (unpatched, dist >= PATCH_T): u16 quant <=1.1e-2, R fp16 4.9e-4,
G fp16 4.9e-4, out bf16 2e-3, eps-skip <=4e-4, ACT spline ~2e-4 => ~1.4e-2
worst-element (gate 2e-2); L2 ~1e-3.
"""

import sys

for _p in ("/opt/trn_rl_repo",):
    if _p not in sys.path:
        sys.path.insert(0, _p)

import ml_dtypes
import numpy as np

import concourse.bacc as bacc
import concourse.dve_ops as dve_ops
import concourse.mybir as mybir
import concourse.tile as tile
from concourse.bass_utils import run_bass_kernel_spmd
from concourse.dve_spec import Spec, Src0, Src1, C0, C1, lower, sq
from concourse.dve_uop import DveOpSpec


def _register(name, spec, subdim=False):
    if name in dve_ops._SUB_OPCODE_FOR_NAME:
        return next(op for op in dve_ops.OPS if op.name == name)
    op = dve_ops.DveOp(name, spec, subdim=subdim, uops_sha={})
    dve_ops.OPS.append(op)
    dve_ops._SUB_OPCODE_FOR_NAME[name] = (
        dve_ops._CUSTOM_DVE_ROW_BASE + len(dve_ops.OPS) - 1
    )
    dve_ops.CUSTOM_DVE_SPECS[name] = spec
    for ver in ("v3", "v4"):
        s = DveOpSpec(
            name=name,
            opcode=dve_ops.get_dve_sub_opcode(name),
            uops=lower(spec, ver=ver),
            rd1_en=dve_ops.has_src1(spec),
        )
        op.uops_sha[ver] = s.sha(ver)
    return op


# S = (x + cx)^2 + (y + cy)^2   (cx, cy per-partition APs; x,y u16 streams)
DIST2 = _register(
    "DIST2_AFF_ANT",
    Spec(
        body=sq(Src0 + C0) + sq(Src1 + C1),
        reference=lambda in0, in1, s0, s1, imm2: (
            (in0.astype(np.float32) + s0) ** 2 + (in1.astype(np.float32) + s1) ** 2
        ).astype(np.float32),
    ),
)

NUM_CORES = 8
EPS = 1e-6
P = 128
WP = 7824  # per-partition elements per core (padded)
FC = 1_000_000
RPAD = P * WP  # 1,001,472
PATCH_T = 2.5e-3  # host recomputes rows with dist(robot) < PATCH_T exactly
# sub-tile schedule: small edge pieces shorten pipeline fill and drain.
# All widths even => every bitcast sub-block is 4B-aligned (TT 2x mode).
# Growth-rate-matched schedule: V consumes 3.125ns/col vs DMA delivering
# 2.13ns/col, so step k+1 may be at most ~1.47x step k or the DVE starves
# during the ramp (per-step DMA sem latency ~0.9us is pipelined away only
# when the NEXT transfer fits inside the CURRENT compute window).
WIDTHS = (120, 344, 672, 1154, 1400, 1400, 1400, 1000, 334)
assert sum(WIDTHS) == WP and all(w % 2 == 0 for w in WIDTHS)
OUT_LAG = 4  # defer each step's output DMA ~4 steps: inputs get DMA priority
# early (V never starves), outputs drain into the V-bound tail when the
# input stream is exhausted and the DMA rings would otherwise idle.

_nc_cache = None


def _act_raw(nc, out_ap, in_ap, func, scale=1.0, bias=None):
    """Emit an activation directly (bypasses the Rsqrt accuracy guard --
    measured ~5e-4 rel err on TRN2, fine for this kernel's 2e-2 gate)."""
    if bias is None:
        bias = nc.const_aps.scalar_like(0.0, in_ap)
    ins = [
        nc.scalar.lower_ap(in_ap),
        nc.scalar.lower_ap(bias),
        mybir.ImmediateValue(dtype=mybir.dt.float32, value=float(scale)),
        mybir.ImmediateValue(dtype=mybir.dt.float32, value=0.0),
    ]
    return nc.scalar.add_instruction(
        mybir.InstActivation(
            name=nc.get_next_instruction_name(),
            func=func,
            ins=ins,
            outs=[nc.scalar.lower_ap(out_ap)],
        )
    )


def _build():
    global _nc_cache
    if _nc_cache is not None:
        return _nc_cache

    f32 = mybir.dt.float32
    fp16 = mybir.dt.float16
    bf16 = mybir.dt.bfloat16
    u16 = mybir.dt.uint16
    u8 = mybir.dt.uint8
    mult = mybir.AluOpType.mult
    Rsqrt = mybir.ActivationFunctionType.Rsqrt

    nc = bacc.Bacc(
        "TRN2", target_bir_lowering=False, debug=False, num_devices=NUM_CORES
    )
    # xg: per step block [x u16 (2w B) | y u16 (2w B) | g fp16 (2w B)] at 6a
    xg_ext = nc.declare_dram_parameter("xg", [P, 6 * WP], u8, isOutput=False)
    rb_ext = nc.declare_dram_parameter("rb", [P, 8], f32, isOutput=False)
    out_ext = nc.declare_dram_parameter("out", [P, 2 * WP], bf16, isOutput=True)

    with tile.TileContext(nc) as tc:
        with (
            tc.tile_pool(name="const", bufs=1) as cpool,
            tc.tile_pool(name="io", bufs=6) as io,
            tc.tile_pool(name="wk", bufs=4) as wk,
            tc.tile_pool(name="op", bufs=7) as op,
        ):
            RB = cpool.tile([P, 8], f32)
            # RB rides the ACT ring so the first xg DMA is first on the
            # sync ring (shaves the serialized 0.6us issue ahead of it)
            nc.scalar.dma_start(RB[:], rb_ext[:])
            # warm the ACT rsqrt table so the load overlaps the first data DMA
            warm = cpool.tile([P, 1], f32)
            _act_raw(nc, warm[:], warm[:], Rsqrt)

            pending = []  # (col_a, col_b, O_tile) awaiting deferred out-DMA
            a = 0
            for w in WIDTHS:
                b = a + w
                XG = io.tile([P, 6 * w], u8, tag="xg")
                nc.sync.dma_start(XG[:], xg_ext[:, 6 * a : 6 * b])
                X = XG[:, : 2 * w].bitcast(u16)           # [P, w]
                Y = XG[:, 2 * w : 4 * w].bitcast(u16)     # [P, w]
                G = XG[:, 4 * w :].bitcast(fp16)          # [P, w]

                SS = wk.tile([P, 2 * w], f32, tag="ss")
                nc.vector._custom_dve(
                    DIST2, out=SS[:, :w], in0=X, in1=Y,
                    s0=RB[:, 0:1], s1=RB[:, 2:3],
                )
                nc.vector._custom_dve(
                    DIST2, out=SS[:, w:], in0=X, in1=Y,
                    s0=RB[:, 1:2], s1=RB[:, 3:4],
                )
                R = wk.tile([P, 2 * w], fp16, tag="r")
                # R = Rsqrt(S * 2^-32) = 65536/sqrt(S) = 1/dist; one ACT pass
                # covers both robots' S halves.
                _act_raw(nc, R[:], SS[:], Rsqrt, scale=2.0 ** -32)

                O = op.tile([P, 2 * w], bf16, tag="o")
                nc.vector.tensor_tensor(O[:, :w], R[:, :w], G, mult)
                nc.vector.tensor_tensor(O[:, w:], R[:, w:], G, mult)
                pending.append((a, b, O))
                if len(pending) > OUT_LAG:
                    pa, pb, PO = pending.pop(0)
                    nc.scalar.dma_start(out_ext[:, 2 * pa : 2 * pb], PO[:])
                a = b
            for i, (pa, pb, PO) in enumerate(pending):
                if i == len(pending) - 1:
                    # split the final out so its first half departs while the
                    # second robot's multiply is still in flight
                    pw = pb - pa
                    nc.scalar.dma_start(out_ext[:, 2 * pa : 2 * pa + pw], PO[:, :pw])
                    nc.scalar.dma_start(out_ext[:, 2 * pa + pw : 2 * pb], PO[:, pw:])
                else:
                    nc.scalar.dma_start(out_ext[:, 2 * pa : 2 * pb], PO[:])
    nc.compile()
    _nc_cache = nc
    return nc


def _plane(col, pad, dtype):
    full = np.empty(RPAD, dtype=dtype)
    full[:FC] = col
    full[FC:] = pad
    return full.reshape(P, WP)


def _prepare_in_maps(node_features: np.ndarray):
    nf = np.asarray(node_features, dtype=np.float32)
    robots = nf[:2, :2]  # (2, 2): [robot, (x, y)]
    # pad position: the u16-grid corner farthest from both robots (keeps
    # Rsqrt well away from 0 on pad rows; pad gain = 0 zeroes the output)
    corners = np.array([[0, 0], [0, 65535], [65535, 0], [65535, 65535]], dtype=np.float32)
    d = np.min(
        np.linalg.norm(corners[:, None, :] / 65536.0 - robots[None, :, :], axis=-1),
        axis=1,
    )
    cx, cy = corners[int(np.argmax(d))].astype(np.uint16)

    rb = np.tile(
        np.array(
            [
                -robots[0, 0] * 65536.0, -robots[1, 0] * 65536.0,
                -robots[0, 1] * 65536.0, -robots[1, 1] * 65536.0,
                0.0, 0.0, 0.0, 0.0,
            ],
            dtype=np.float32,
        ),
        (P, 1),
    )
    in_maps = []
    for c in range(NUM_CORES):
        rows = nf[2 + c * FC : 2 + (c + 1) * FC]
        xq = np.clip(np.rint(rows[:, 0] * 65536.0), 0, 65535).astype(np.uint16)
        yq = np.clip(np.rint(rows[:, 1] * 65536.0), 0, 65535).astype(np.uint16)
        g32 = rows[:, 2] + rows[:, 4] + np.float32(EPS)
        x = _plane(xq, cx, np.uint16)
        y = _plane(yq, cy, np.uint16)
        g = _plane(g32.astype(np.float16), 0, np.float16)
        xg = np.empty((P, 6 * WP), dtype=np.uint8)
        a = 0
        for w in WIDTHS:
            b = a + w
            blk = xg[:, 6 * a : 6 * b]
            blk[:, : 2 * w] = x[:, a:b].view(np.uint8)
            blk[:, 2 * w : 4 * w] = y[:, a:b].view(np.uint8)
            blk[:, 4 * w :] = g[:, a:b].view(np.uint8)
            a = b
        in_maps.append({"xg": xg, "rb": rb})
    return in_maps


def _assemble(results) -> np.ndarray:
    a0 = np.empty(NUM_CORES * FC, dtype=np.float32)
    a1 = np.empty(NUM_CORES * FC, dtype=np.float32)
    p0 = np.empty((P, WP), dtype=np.float32)
    p1 = np.empty((P, WP), dtype=np.float32)
    for c in range(NUM_CORES):
        o = np.asarray(results[c]["out"])  # [P, 2*WP] bf16, per-step packed
        a = 0
        for w in WIDTHS:
            b = a + w
            p0[:, a:b] = o[:, 2 * a : 2 * a + w]
            p1[:, a:b] = o[:, 2 * a + w : 2 * b]
            a = b
        a0[c * FC : (c + 1) * FC] = p0.reshape(RPAD)[:FC]
        a1[c * FC : (c + 1) * FC] = p1.reshape(RPAD)[:FC]
    return np.stack([a0, a1], axis=0)


def _patch(nf: np.ndarray, out: np.ndarray) -> None:
    """Exact recompute (f32, matching the reference) for rows near a robot,
    where u16 position quantization would exceed the rel-err gate."""
    fr = nf[2:]
    fx, fy = fr[:, 0], fr[:, 1]
    for r in range(2):
        rx, ry = nf[r, 0], nf[r, 1]
        sel = (np.abs(fx - rx) < PATCH_T) & (np.abs(fy - ry) < PATCH_T)
        idx = np.nonzero(sel)[0]
        if idx.size == 0:
            continue
        dx = fx[idx] - rx
        dy = fy[idx] - ry
        dist = np.sqrt(dx * dx + dy * dy) + np.float32(EPS)
        gain = fr[idx, 2] + fr[idx, 4] + np.float32(EPS)
        out[r, idx] = gain / dist


def run(node_features, trace: bool = False):
    """Returns (affinity, BassKernelResults)."""
    nc = _build()
    nf = np.asarray(node_features, dtype=np.float32)
    in_maps = _prepare_in_maps(nf)
    res = run_bass_kernel_spmd(nc, in_maps, list(range(NUM_CORES)), trace=trace)
    out = _assemble(res.results)
    _patch(nf, out)
    return out, res


def kernel(node_features, edge_features=None, edge_indices=None):
    affinity, _ = run(node_features, trace=False)
    return affinity
